# revision 2
# baseline (speedup 1.0000x reference)
"""Grouped-Query Attention (16 q heads, 4 kv heads, head_dim 128, seq 4096,
hidden 2048) on 8 Trainium2 NeuronCores — bf16, collective-free.

Sharding: sequence-parallel over query tokens (512 per core). Every core
computes the FULL K^T and V from the full (replicated) x — ~190us of extra
bf16 PE time buys zero collectives (slow/fragile on this runtime). K^T and V
stay SBUF-resident; no DRAM round-trip, no AllGather.

All matmuls run in bf16 (1 cycle/row vs 4 for fp32; rel-err budget is 2e-2,
bf16 with fp32 PSUM accumulation lands ~1e-3). Weights and x are cast to bf16
on the host. Softmax without max-subtraction (|scores| ~ 3): scores are built
transposed S^T[k, q], exp runs on the scalar engine out of PSUM writing bf16,
Z = sum_k exp via a ones-vector bf16 matmul accumulating in PSUM, and the
1/Z normalization is a ones-broadcast matmul + DVE multiply into bf16 attT.
"""

import numpy as np

import concourse.bass as bass
import concourse.bacc as bacc
import concourse.tile as tile
from concourse import mybir
from concourse.bass_utils import run_bass_kernel_spmd

# Problem constants
S = 4096          # sequence length
HID = 2048        # hidden dim
NH = 16           # query heads
NKV = 4           # kv heads
D = 128           # head dim
G = NH // NKV     # q heads per kv head (4)
NC = 8            # cores
SC = S // NC      # query tokens per core (512)
P = 128           # partitions
KT = HID // P     # contraction tiles over hidden (16)
SK = S // P       # key tiles (32)
CB = 512          # kv-projection column block (seq positions)
NCB = S // CB     # 8 col blocks
INV_NORM = 1.0 / float(np.sqrt(D))

FP = mybir.dt.float32
BF = mybir.dt.bfloat16


def build_bass():
    nc = bacc.Bacc(None, num_devices=NC)

    # ---- I/O (bf16 inputs, host-prepped; fp32 output) ----
    xT = nc.declare_dram_parameter("xT", [KT, P, S], BF, isOutput=False)
    xq = nc.declare_dram_parameter("xq", [KT, P, SC], BF, isOutput=False)
    wkv = nc.declare_dram_parameter("wkv", [KT, P, 2 * NKV * D], BF, isOutput=False)
    wq = nc.declare_dram_parameter("wq", [NH, KT, P, D], BF, isOutput=False)
    wo = nc.declare_dram_parameter("wo", [2 * KT, P, HID // 2], BF, isOutput=False)
    y = nc.declare_dram_parameter("y", [SC, HID], FP, isOutput=True)

    with tile.TileContext(nc) as tc:
        with (
            tc.tile_pool(name="const", bufs=1) as const_pool,
            tc.tile_pool(name="persist", bufs=1) as pp,
        ):
            ones_k = const_pool.tile([P, 1], BF)       # Z-sum lhsT
            nc.vector.memset(ones_k[:], 1.0)
            ones_m = const_pool.tile([1, P], FP)       # 1/Z broadcast lhsT (K=1)
            nc.vector.memset(ones_m[:], 1.0)

            # persistent SBUF (KB/partition): kT 32 + v 32 + qT 16 + attT 16
            # + xq 16 = 112 of ~207 usable; wkv(32)+x-stream(32) live only in
            # phase 1a, wo(64) loads after they are freed.
            kT_sb = pp.tile([P, NKV, S], BF)
            v_sb = pp.tile([P, NKV, SK, D], BF)
            qT_sb = pp.tile([P, NH, SC], BF)
            attT_sb = pp.tile([P, NH, SC], BF)
            xq_sb = pp.tile([P, KT, SC], BF)

            for h in range(KT):
                nc.sync.dma_start(out=xq_sb[:, h, :], in_=xq[h])

            # ---------- Phase 1a: full K^T and V projections ----------
            with (
                tc.tile_pool(name="wkv_sb", bufs=1) as wkv_pool,
                tc.tile_pool(name="xcb", bufs=2) as x_pool,
                tc.tile_pool(name="kt_psum", bufs=2, space="PSUM") as kt_psum,
                tc.tile_pool(name="v_psum", bufs=3, space="PSUM") as v_psum,
            ):
                wkv_sb = wkv_pool.tile([P, KT, 2 * NKV * D], BF)
                for h in range(KT):
                    nc.sync.dma_start(out=wkv_sb[:, h, :], in_=wkv[h])
                for cb in range(NCB):
                    xcb = x_pool.tile([P, KT, CB], BF)
                    for h in range(KT):
                        nc.sync.dma_start(
                            out=xcb[:, h, :], in_=xT[h, :, cb * CB:(cb + 1) * CB]
                        )
                    # K^T block: [NKV*D, CB]
                    for o in range(NKV):
                        ps = kt_psum.tile([P, CB], FP)
                        for h in range(KT):
                            nc.tensor.matmul(
                                ps[:],
                                wkv_sb[:, h, o * D:(o + 1) * D],
                                xcb[:, h, :],
                                start=(h == 0), stop=(h == KT - 1),
                            )
                        nc.vector.tensor_copy(
                            kT_sb[:, o, cb * CB:(cb + 1) * CB], ps[:]
                        )
                    # V block (natural): keys on partitions
                    for st in range(CB // P):
                        ps = v_psum.tile([P, NKV * D], FP)
                        for h in range(KT):
                            nc.tensor.matmul(
                                ps[:],
                                xcb[:, h, st * P:(st + 1) * P],
                                wkv_sb[:, h, NKV * D:],
                                start=(h == 0), stop=(h == KT - 1),
                            )
                        sk = cb * (CB // P) + st
                        nc.vector.tensor_copy(v_sb[:, :, sk, :], ps[:])

            # ---------- Phases 1b+2+3 share the wo preload pool ----------
            with tc.tile_pool(name="wo_sb", bufs=1) as wo_pool:
                wo_sb = wo_pool.tile([P, 2 * KT, HID // 2], BF)
                # overlaps with q-projection / attention
                for h in range(2 * KT):
                    nc.sync.dma_start(out=wo_sb[:, h, :], in_=wo[h])

                def q_proj(h, wq_pool, q_psum):
                    wqo = wq_pool.tile([P, KT, D], BF, tag="wqo")
                    for k in range(KT):
                        nc.sync.dma_start(out=wqo[:, k, :], in_=wq[h, k])
                    ps = q_psum.tile([P, SC], FP, name="qps", tag="stp")
                    for k in range(KT):
                        nc.tensor.matmul(
                            ps[:], wqo[:, k, :], xq_sb[:, k, :],
                            start=(k == 0), stop=(k == KT - 1),
                        )
                    nc.vector.tensor_copy(qT_sb[:, h, :], ps[:])

                with (
                    tc.tile_pool(name="wq_sb", bufs=2) as wq_pool,
                    tc.tile_pool(name="st_psum", bufs=3, space="PSUM") as st_psum,
                    tc.tile_pool(name="av_psum", bufs=2, space="PSUM") as av_psum,
                    tc.tile_pool(name="z_psum", bufs=2, space="PSUM") as z_psum,
                    tc.tile_pool(name="bc_psum", bufs=1, space="PSUM") as bc_psum,
                    tc.tile_pool(name="p_sb", bufs=4) as p_pool,
                    tc.tile_pool(name="z_sb", bufs=2) as zs_pool,
                ):
                    # q heads for group 0 first so attention starts early
                    for h in range(G):
                        q_proj(h, wq_pool, st_psum)

                    for g in range(NKV):
                        for hl in range(G):
                            h = g * G + hl
                            av = av_psum.tile([P, SC], FP, tag="av")
                            zp = z_psum.tile([1, SC], FP, tag="zp")
                            for sk in range(SK):
                                stp = st_psum.tile([P, SC], FP, tag="stp")
                                nc.tensor.matmul(
                                    stp[:],
                                    kT_sb[:, g, sk * P:(sk + 1) * P],
                                    qT_sb[:, h, :],
                                    start=True, stop=True,
                                )
                                ptile = p_pool.tile([P, SC], BF)
                                nc.scalar.activation(
                                    ptile[:], stp[:],
                                    mybir.ActivationFunctionType.Exp,
                                    scale=INV_NORM,
                                )
                                nc.tensor.matmul(
                                    zp[:], ones_k[:], ptile[:],
                                    start=(sk == 0), stop=(sk == SK - 1),
                                )
                                nc.tensor.matmul(
                                    av[:], v_sb[:, g, sk, :], ptile[:],
                                    start=(sk == 0), stop=(sk == SK - 1),
                                )
                            zr = zs_pool.tile([1, SC], FP, tag="zr")
                            nc.vector.reciprocal(zr[:], zp[:])
                            bc = bc_psum.tile([P, SC], FP)
                            nc.tensor.matmul(
                                bc[:], ones_m[:], zr[:], start=True, stop=True,
                            )
                            bcs = zs_pool.tile([P, SC], FP, tag="bcs")
                            nc.vector.tensor_copy(bcs[:], bc[:])
                            nc.vector.tensor_mul(attT_sb[:, h, :], av[:], bcs[:])
                        # next group's q heads while attention above drains
                        if g < NKV - 1:
                            for h in range((g + 1) * G, (g + 2) * G):
                                q_proj(h, wq_pool, st_psum)

                # ---------- Phase 3: output projection ----------
                with (
                    tc.tile_pool(name="y_psum", bufs=4, space="PSUM") as y_psum,
                    tc.tile_pool(name="y_sb", bufs=3) as ys_pool,
                ):
                    NW = 512  # PSUM-bank-sized output chunk
                    for ma in range(SC // P):       # 4 query-row tiles
                        ysb = ys_pool.tile([P, HID], FP)
                        for n in range(HID // NW):  # 4 output-column chunks
                            half, col = n // 2, (n % 2) * NW
                            ps = y_psum.tile([P, NW], FP, name="yp", tag="yp")
                            for k in range(KT):
                                nc.tensor.matmul(
                                    ps[:],
                                    attT_sb[:, k, ma * P:(ma + 1) * P],
                                    wo_sb[:, half * KT + k, col:col + NW],
                                    start=(k == 0), stop=(k == KT - 1),
                                )
                            nc.vector.tensor_copy(
                                ysb[:, n * NW:(n + 1) * NW], ps[:])
                        nc.sync.dma_start(
                            out=y[ma * P:(ma + 1) * P, :], in_=ysb[:])
    nc.compile()
    return nc


_CACHED = {}


def _prep_inputs(x, Wq, Wk, Wv, Wo):
    bf16 = mybir.dt.np(BF)
    xs = np.ascontiguousarray(x.reshape(S, HID)).astype(np.float32)
    xT_flat = xs.T.astype(bf16)                          # [HID, S]
    xT_t = np.ascontiguousarray(xT_flat.reshape(KT, P, S))
    wkT = Wk.T.astype(bf16)                              # [HID, NKV*D]
    wvT = Wv.T.astype(bf16)
    wkv_t = np.ascontiguousarray(
        np.concatenate([wkT, wvT], axis=1).reshape(KT, P, 2 * NKV * D))
    wqT = Wq.T.astype(bf16)                              # [HID, NH*D]
    wq_t = np.empty((NH, KT, P, D), bf16)
    for o in range(NH):
        for h in range(KT):
            wq_t[o, h] = wqT[h * P:(h + 1) * P, o * D:(o + 1) * D]
    woT = Wo.T.astype(bf16)                              # [HID(contract), HID(out)]
    wo_t = np.empty((2 * KT, P, HID // 2), bf16)
    for half in range(2):
        for k in range(KT):
            wo_t[half * KT + k] = woT[k * P:(k + 1) * P,
                                      half * (HID // 2):(half + 1) * (HID // 2)]
    in_maps = []
    for c in range(NC):
        xq_c = np.ascontiguousarray(
            xT_flat[:, c * SC:(c + 1) * SC].reshape(KT, P, SC))
        in_maps.append({
            "xT": xT_t, "xq": xq_c, "wkv": wkv_t, "wq": wq_t, "wo": wo_t,
        })
    return in_maps


def run(x, Wq, Wk, Wv, Wo, trace=False):
    if "nc" not in _CACHED:
        _CACHED["nc"] = build_bass()
    nc = _CACHED["nc"]
    in_maps = _prep_inputs(x, Wq, Wk, Wv, Wo)
    res = run_bass_kernel_spmd(nc, in_maps, list(range(NC)), trace=trace)
    out = np.concatenate([res.results[c]["y"] for c in range(NC)], axis=0)
    return out.reshape(1, S, HID), res


def kernel(x, Wq, Wk, Wv, Wo):
    out, _ = run(np.asarray(x), np.asarray(Wq), np.asarray(Wk),
                 np.asarray(Wv), np.asarray(Wo))
    return out


# revision 3
# speedup vs baseline: 2.4988x; 2.4988x over previous
"""Grouped-Query Attention (16 q heads, 4 kv heads, head_dim 128, seq 4096,
hidden 2048) on 8 Trainium2 NeuronCores — bf16, collective-free.

Sharding: sequence-parallel over query tokens (512 per core). Every core
computes the FULL K^T and V from the full (replicated) x — ~190us of extra
bf16 PE time buys zero collectives (slow/fragile on this runtime). K^T and V
stay SBUF-resident; no DRAM round-trip, no AllGather.

All matmuls run in bf16 (1 PE cycle/row vs 4 for fp32; rel-err lands ~4.3e-3
against the 2e-2 budget, fp32 PSUM accumulation). Weights and x are cast to
bf16 on the host. Softmax without max-subtraction (|scores| ~ 3): scores are
built transposed S^T[k, q], exp runs on the scalar engine out of PSUM writing
bf16, the softmax denominator accumulates on the gpsimd (Pool) engine via
ping-pong adds (keeping it off the busier PE/DVE), and 1/Z normalization is a
ones-broadcast matmul + DVE multiply into bf16 attT.

Schedule: q-head projections, the wo weight stream and the first half of the
output projection are spread one-per-head-boundary through the Act-bound
attention phase, so the scalar engine's exp backlog absorbs them; the output
projection runs in two waves (heads 0-7 into an SBUF partial during the
attention of heads 8-15, heads 8-15 + partial -> y in the tail).

Timing-sim: ~674us/core vs 2062us for the staged fp32+AllGather baseline.
"""

import numpy as np

import concourse.bass as bass
import concourse.bacc as bacc
import concourse.tile as tile
from concourse import mybir
from concourse.bass_utils import run_bass_kernel_spmd

# Problem constants
S = 4096          # sequence length
HID = 2048        # hidden dim
NH = 16           # query heads
NKV = 4           # kv heads
D = 128           # head dim
G = NH // NKV     # q heads per kv head (4)
NC = 8            # cores
SC = S // NC      # query tokens per core (512)
P = 128           # partitions
KT = HID // P     # contraction tiles over hidden (16)
SK = S // P       # key tiles (32)
CB = 512          # kv-projection column block (seq positions)
NCB = S // CB     # 8 col blocks
INV_NORM = 1.0 / float(np.sqrt(D))

FP = mybir.dt.float32
BF = mybir.dt.bfloat16


def build_bass():
    nc = bacc.Bacc(None, num_devices=NC)

    # ---- I/O (bf16 inputs, host-prepped; fp32 output) ----
    xT = nc.declare_dram_parameter("xT", [KT, P, S], BF, isOutput=False)
    xq = nc.declare_dram_parameter("xq", [KT, P, SC], BF, isOutput=False)
    wkv = nc.declare_dram_parameter("wkv", [KT, P, 2 * NKV * D], BF, isOutput=False)
    wq = nc.declare_dram_parameter("wq", [NH, KT, P, D], BF, isOutput=False)
    wo = nc.declare_dram_parameter("wo", [2 * KT, P, HID // 2], BF, isOutput=False)
    y = nc.declare_dram_parameter("y", [SC, HID], FP, isOutput=True)

    with tile.TileContext(nc) as tc:
        with (
            tc.tile_pool(name="const", bufs=1) as const_pool,
            tc.tile_pool(name="persist", bufs=1) as pp,
        ):
            ones_kf = const_pool.tile([P, 1], FP)      # Z-sum lhsT (fp32)
            nc.vector.memset(ones_kf[:], 1.0)
            ones_m = const_pool.tile([1, P], FP)       # 1/Z broadcast lhsT (K=1)
            nc.vector.memset(ones_m[:], 1.0)

            # persistent SBUF (KB/partition): kT 32 + v 32 + qT 16 + attT 16
            # + xq 16 = 112 of ~207 usable; wkv(32)+x-stream(32) live only in
            # phase 1a, wo(64) loads after they are freed.
            kT_sb = pp.tile([P, NKV, S], BF)
            v_sb = pp.tile([P, NKV, SK, D], BF)
            qT_sb = pp.tile([P, NH, SC], BF)
            attT_sb = pp.tile([P, NH, SC], BF)
            xq_sb = pp.tile([P, KT, SC], BF)

            # ---------- Phase 1a: full K^T and V projections ----------
            with (
                tc.tile_pool(name="wkv_sb", bufs=1) as wkv_pool,
                tc.tile_pool(name="xcb", bufs=2) as x_pool,
                tc.tile_pool(name="kt_psum", bufs=2, space="PSUM") as kt_psum,
                tc.tile_pool(name="v_psum", bufs=3, space="PSUM") as v_psum,
            ):
                # wkv + first x block gate the first matmul — load them first
                wkv_sb = wkv_pool.tile([P, KT, 2 * NKV * D], BF)
                for h in range(KT):
                    nc.sync.dma_start(out=wkv_sb[:, h, :], in_=wkv[h])
                for cb in range(NCB):
                    xcb = x_pool.tile([P, KT, CB], BF)
                    for h in range(KT):
                        nc.sync.dma_start(
                            out=xcb[:, h, :], in_=xT[h, :, cb * CB:(cb + 1) * CB]
                        )
                    if cb == 0:
                        for h in range(KT):
                            nc.sync.dma_start(out=xq_sb[:, h, :], in_=xq[h])
                    # K^T block: [NKV*D, CB]
                    for o in range(NKV):
                        ps = kt_psum.tile([P, CB], FP)
                        for h in range(KT):
                            nc.tensor.matmul(
                                ps[:],
                                wkv_sb[:, h, o * D:(o + 1) * D],
                                xcb[:, h, :],
                                start=(h == 0), stop=(h == KT - 1),
                            )
                        nc.vector.tensor_copy(
                            kT_sb[:, o, cb * CB:(cb + 1) * CB], ps[:]
                        )
                    # V block (natural): keys on partitions
                    for st in range(CB // P):
                        ps = v_psum.tile([P, NKV * D], FP)
                        for h in range(KT):
                            nc.tensor.matmul(
                                ps[:],
                                xcb[:, h, st * P:(st + 1) * P],
                                wkv_sb[:, h, NKV * D:],
                                start=(h == 0), stop=(h == KT - 1),
                            )
                        sk = cb * (CB // P) + st
                        nc.vector.tensor_copy(v_sb[:, :, sk, :], ps[:])

            # ---------- Phases 1b+2+3 ----------
            with tc.tile_pool(name="wo_t", bufs=2) as wo_pool:

                def q_proj(h, wq_pool, q_psum):
                    wqo = wq_pool.tile([P, KT, D], BF, tag="wqo")
                    for k in range(KT):
                        nc.sync.dma_start(out=wqo[:, k, :], in_=wq[h, k])
                    ps = q_psum.tile([P, SC], FP, name="qps", tag="stp")
                    for k in range(KT):
                        nc.tensor.matmul(
                            ps[:], wqo[:, k, :], xq_sb[:, k, :],
                            start=(k == 0), stop=(k == KT - 1),
                        )
                    nc.vector.tensor_copy(qT_sb[:, h, :], ps[:])

                with (
                    tc.tile_pool(name="wq_sb", bufs=2) as wq_pool,
                    tc.tile_pool(name="st_psum", bufs=4, space="PSUM") as st_psum,
                    tc.tile_pool(name="av_psum", bufs=2, space="PSUM") as av_psum,
                    tc.tile_pool(name="bc_psum", bufs=1, space="PSUM") as bc_psum,
                    tc.tile_pool(name="y_psum", bufs=1, space="PSUM") as y_psum,
                    tc.tile_pool(name="p_sb", bufs=6) as p_pool,
                    tc.tile_pool(name="pacc_sb", bufs=2) as pacc_pool,
                    tc.tile_pool(name="z_sb", bufs=2) as zs_pool,
                    tc.tile_pool(name="ypart_sb", bufs=1) as yp_pool,
                    tc.tile_pool(name="y_sb", bufs=2) as ys_pool,
                ):
                    y_part = yp_pool.tile([P, SC // P, HID], FP)  # 32 KB/part
                    wo_cur = [None]

                    def out_proj_unit(u, wave):
                        # wave 0: heads 0-7 -> y_part (runs inside the
                        # Act-bound attention of heads 8-15); wave 1: heads
                        # 8-15 + y_part -> y.  wo streamed per column chunk n,
                        # reused across the 4 query-row tiles ma.
                        n, ma = u // 4, u % 4
                        half, col = n // 2, (n % 2) * 512
                        if ma == 0:
                            wt = wo_pool.tile([P, 8, 512], BF, name="wt", tag="wt")
                            for k in range(8):
                                nc.sync.dma_start(
                                    out=wt[:, k, :],
                                    in_=wo[half * KT + 8 * wave + k,
                                           :, col:col + 512])
                            wo_cur[0] = wt
                        wt = wo_cur[0]
                        ps = y_psum.tile([P, 512], FP, name="yps", tag="yps")
                        for k in range(8):
                            nc.tensor.matmul(
                                ps[:],
                                attT_sb[:, 8 * wave + k, ma * P:(ma + 1) * P],
                                wt[:, k, :],
                                start=(k == 0), stop=(k == 7),
                            )
                        if wave == 0:
                            nc.vector.tensor_copy(
                                y_part[:, ma, n * 512:(n + 1) * 512], ps[:])
                        else:
                            ysb = ys_pool.tile([P, 512], FP)
                            nc.vector.tensor_add(
                                ysb[:], y_part[:, ma, n * 512:(n + 1) * 512],
                                ps[:])
                            nc.sync.dma_start(
                                out=y[ma * P:(ma + 1) * P, n * 512:(n + 1) * 512],
                                in_=ysb[:])
                    # only head 0's projection gates the first score matmul
                    q_proj(0, wq_pool, st_psum)

                    for g in range(NKV):
                        for hl in range(G):
                            h = g * G + hl
                            # spread the remaining q-projs, the wo preload and
                            # the first out-proj wave across head boundaries:
                            # the Act exp backlog absorbs these PE/DMA inserts
                            if g == 0:
                                q_proj(1 + hl, wq_pool, st_psum)
                                if 5 + hl < 2 * G:
                                    q_proj(5 + hl, wq_pool, st_psum)
                            elif g == 1:
                                q_proj(2 * G + 2 * hl, wq_pool, st_psum)
                                q_proj(2 * G + 2 * hl + 1, wq_pool, st_psum)
                            else:
                                ci = ((g - 2) * G + hl) * 2
                                out_proj_unit(ci, 0)
                                out_proj_unit(ci + 1, 0)
                            av = av_psum.tile([P, SC], FP, tag="av")
                            # Z = sum_k exp: accumulated on DVE (ping-pong, no
                            # in-place) to keep the 32 ones-matmuls off the PE
                            pacc = [
                                pacc_pool.tile([P, SC], FP, name="pacc0", tag="pacc0"),
                                pacc_pool.tile([P, SC], FP, name="pacc1", tag="pacc1"),
                            ]
                            for sk in range(SK):
                                stp = st_psum.tile([P, SC], FP, tag="stp")
                                nc.tensor.matmul(
                                    stp[:],
                                    kT_sb[:, g, sk * P:(sk + 1) * P],
                                    qT_sb[:, h, :],
                                    start=True, stop=True,
                                )
                                ptile = p_pool.tile([P, SC], BF)
                                nc.scalar.activation(
                                    ptile[:], stp[:],
                                    mybir.ActivationFunctionType.Exp,
                                    scale=INV_NORM,
                                )
                                if sk == 0:
                                    nc.gpsimd.tensor_copy(pacc[0][:], ptile[:])
                                else:
                                    nc.gpsimd.tensor_add(
                                        pacc[sk % 2][:], pacc[(sk + 1) % 2][:],
                                        ptile[:],
                                    )
                                nc.tensor.matmul(
                                    av[:], v_sb[:, g, sk, :], ptile[:],
                                    start=(sk == 0), stop=(sk == SK - 1),
                                )
                            # Z lands in row 0 of the bc tile (same bank),
                            # read out by the reciprocal before the broadcast
                            # matmul resets the whole tile
                            bc = bc_psum.tile([P, SC], FP)
                            nc.tensor.matmul(
                                bc[0:1, :], ones_kf[:], pacc[(SK - 1) % 2][:],
                                start=True, stop=True,
                            )
                            zr = zs_pool.tile([1, SC], FP, tag="zr")
                            nc.vector.reciprocal(zr[:], bc[0:1, :])
                            nc.tensor.matmul(
                                bc[:], ones_m[:], zr[:], start=True, stop=True,
                            )
                            bcs = zs_pool.tile([P, SC], FP, tag="bcs")
                            nc.vector.tensor_copy(bcs[:], bc[:])
                            nc.vector.tensor_mul(attT_sb[:, h, :], av[:], bcs[:])

                    # ---------- out-proj wave 1 (tail) ----------
                    for u in range(16):
                        out_proj_unit(u, 1)
    nc.compile()
    return nc


_CACHED = {}


def _prep_inputs(x, Wq, Wk, Wv, Wo):
    bf16 = mybir.dt.np(BF)
    xs = np.ascontiguousarray(x.reshape(S, HID)).astype(np.float32)
    xT_flat = xs.T.astype(bf16)                          # [HID, S]
    xT_t = np.ascontiguousarray(xT_flat.reshape(KT, P, S))
    wkT = Wk.T.astype(bf16)                              # [HID, NKV*D]
    wvT = Wv.T.astype(bf16)
    wkv_t = np.ascontiguousarray(
        np.concatenate([wkT, wvT], axis=1).reshape(KT, P, 2 * NKV * D))
    wqT = Wq.T.astype(bf16)                              # [HID, NH*D]
    wq_t = np.empty((NH, KT, P, D), bf16)
    for o in range(NH):
        for h in range(KT):
            wq_t[o, h] = wqT[h * P:(h + 1) * P, o * D:(o + 1) * D]
    woT = Wo.T.astype(bf16)                              # [HID(contract), HID(out)]
    wo_t = np.empty((2 * KT, P, HID // 2), bf16)
    for half in range(2):
        for k in range(KT):
            wo_t[half * KT + k] = woT[k * P:(k + 1) * P,
                                      half * (HID // 2):(half + 1) * (HID // 2)]
    in_maps = []
    for c in range(NC):
        xq_c = np.ascontiguousarray(
            xT_flat[:, c * SC:(c + 1) * SC].reshape(KT, P, SC))
        in_maps.append({
            "xT": xT_t, "xq": xq_c, "wkv": wkv_t, "wq": wq_t, "wo": wo_t,
        })
    return in_maps


def run(x, Wq, Wk, Wv, Wo, trace=False):
    if "nc" not in _CACHED:
        _CACHED["nc"] = build_bass()
    nc = _CACHED["nc"]
    in_maps = _prep_inputs(x, Wq, Wk, Wv, Wo)
    res = run_bass_kernel_spmd(nc, in_maps, list(range(NC)), trace=trace)
    out = np.concatenate([res.results[c]["y"] for c in range(NC)], axis=0)
    return out.reshape(1, S, HID), res


def kernel(x, Wq, Wk, Wv, Wo):
    out, _ = run(np.asarray(x), np.asarray(Wq), np.asarray(Wk),
                 np.asarray(Wv), np.asarray(Wo))
    return out
